# revision 1
# baseline (speedup 1.0000x reference)
"""GQA (B=2,S=1024,E=4096,H=32,KV=8,HD=128, RoPE, no causal mask) on 8 NeuronCores.

Sharding: 2 batch-groups x 4-way head tensor-parallel.
Core c: batch b=c//4, tp rank r=c%4 -> 8 q heads [8r,8r+8), 2 kv heads [2r,2r+2),
wo rows [1024r, 1024(r+1)).  Each core computes a partial output
out_part = y_local @ wo[local_rows, :]  (emitted transposed as [4096, 1024]);
host sums the 4 partials per batch. No device collectives needed.
"""
import sys

sys.path.insert(0, "/opt/trn_rl_repo")

import numpy as np

B = 2
S = 1024
E = 4096
HD = 128
N_CORES = 8
TP = 4            # tensor-parallel ranks per batch group
HL = 8            # q heads per core
KVL = 2           # kv heads per core
QCOLS = HL * HD   # 1024
KVCOLS = KVL * HD  # 256
NCC = (QCOLS + 2 * KVCOLS) // 128  # 12 col-chunks of 128 (8 q, 2 k, 2 v)
ECH = E // 128    # 32 e-chunks
TT = S // 128     # 8 token tiles
SCALE = 1.0 / np.sqrt(np.float32(HD))
MM_DT = "float16"   # matmul operand dtype: "float16" or "float32r"


_PROGRAM = None


def _build_program():
    import concourse.bass as bass  # noqa: F401
    from concourse import bacc
    import concourse.mybir as mybir
    from concourse.tile import TileContext
    from concourse.masks import make_identity

    dt = mybir.dt.float32
    dtr = getattr(mybir.dt, MM_DT)
    nc = bacc.Bacc("TRN2", target_bir_lowering=False, debug=False,
                   num_devices=N_CORES)

    xt_d = nc.declare_dram_parameter("xt", [E, S], dtr, isOutput=False)
    wq_d = nc.declare_dram_parameter("wq", [E, QCOLS], dtr, isOutput=False)
    wk_d = nc.declare_dram_parameter("wk", [E, KVCOLS], dtr, isOutput=False)
    wv_d = nc.declare_dram_parameter("wv", [E, KVCOLS], dtr, isOutput=False)
    wo_d = nc.declare_dram_parameter("wo", [QCOLS, E], dtr, isOutput=False)
    cos_d = nc.declare_dram_parameter("cos", [HD, S], dtr, isOutput=False)
    sinp_d = nc.declare_dram_parameter("sinp", [HD, S], dtr, isOutput=False)
    out_d = nc.declare_dram_parameter("out_t", [E, S], dt, isOutput=True)

    with TileContext(nc) as tc:
        with tc.tile_pool(name="const", bufs=1) as cpool, \
             tc.tile_pool(name="persist", bufs=1) as ppool, \
             tc.tile_pool(name="vnat", bufs=1) as vpool:
            ident_f = cpool.tile([128, 128], dt)
            make_identity(nc, ident_f[:])
            ident = cpool.tile([128, 128], dtr)
            nc.scalar.copy(ident[:], ident_f[:])
            ones_f = cpool.tile([128, 1], dt)
            nc.vector.memset(ones_f[:], 1.0)
            cos_t = cpool.tile([HD, S], dtr, tag="cos")
            sinp_t = cpool.tile([HD, S], dtr, tag="sinp")
            # persistent tiles: qkvT[cc] = [128 cols, S] transposed projections
            qkvT = [ppool.tile([128, S], dtr, tag=f"qkvT{i}", name=f"qkvT{i}") for i in range(NCC)]
            # yT[h] = [128 hd, S] transposed attention outputs
            yT = [ppool.tile([128, S], dtr, tag=f"yT{i}", name=f"yT{i}") for i in range(HL)]
            # v natural tiles with ones column: [128 k-tokens, HD+1]
            v_nat = [[vpool.tile([128, HD + 1], dtr, tag=f"v{kv}_{kt}", name=f"v{kv}_{kt}")
                      for kt in range(TT)] for kv in range(KVL)]

            # ---------------- Phase A: QKV^T projections (x^T from host) ----------------
            ECS = 8   # e-chunks per superchunk
            NSUP = ECH // ECS  # 4
            with tc.tile_pool(name="xsup", bufs=2) as xspool, \
                 tc.tile_pool(name="wstream", bufs=3) as wpool, \
                 tc.tile_pool(name="rope", bufs=3) as ropool, \
                 tc.tile_pool(name="psA", bufs=2, space="PSUM") as psA:
                CC_ORDER = [HL, HL + 1] + list(range(HL)) + [HL + KVL, HL + KVL + 1]

                def w_src(es, cc):
                    if cc < HL:
                        return wq_d[es * 1024:(es + 1) * 1024, cc * 128:(cc + 1) * 128]
                    if cc < HL + KVL:
                        return wk_d[es * 1024:(es + 1) * 1024,
                                    (cc - HL) * 128:(cc - HL + 1) * 128]
                    return wv_d[es * 1024:(es + 1) * 1024,
                                (cc - HL - KVL) * 128:(cc - HL - KVL + 1) * 128]

                for es in range(NSUP):
                    # first weight tile before the xs chunks so the first matmul
                    # is not stuck behind 8 queued DMAs
                    wt0 = wpool.tile([128, ECS, 128], dtr, tag="w", name=f"wt0_{es}")
                    nc.sync.dma_start(
                        out=wt0[:],
                        in_=w_src(es, CC_ORDER[0]).rearrange("(c p) m -> p c m", p=128))
                    xs = xspool.tile([128, ECS, S], dtr, tag="xs", name=f"xs{es}")
                    for ec in range(ECS):
                        base = es * 1024 + ec * 128
                        nc.sync.dma_start(
                            out=xs[:, ec, :], in_=xt_d[base:base + 128, :])
                    if es == 1:
                        nc.sync.dma_start(out=cos_t[:], in_=cos_d[:])
                        nc.sync.dma_start(out=sinp_t[:], in_=sinp_d[:])
                    for ci, cc in enumerate(CC_ORDER):
                        if ci == 0:
                            wt = wt0
                        else:
                            wt = wpool.tile([128, ECS, 128], dtr, tag="w")
                            nc.sync.dma_start(
                                out=wt[:],
                                in_=w_src(es, cc).rearrange("(c p) m -> p c m", p=128))
                        acc = psA.tile([128, S], dt, tag="acc")
                        for ec in range(ECS):
                            for tb in range(2):
                                nc.tensor.matmul(
                                    acc[:, tb * 512:(tb + 1) * 512], wt[:, ec, :],
                                    xs[:, ec, tb * 512:(tb + 1) * 512],
                                    start=(ec == 0), stop=(ec == ECS - 1))
                        if es == 0:
                            nc.scalar.copy(qkvT[cc][:], acc[:])
                        else:
                            nc.vector.tensor_add(qkvT[cc][:], acc[:], qkvT[cc][:])
                        if es == NSUP - 1 and cc < HL + KVL:
                            # rope immediately after the final accumulation of
                            # this chunk, overlapping remaining projections
                            sh = ropool.tile([HD, S], dtr, tag="sh")
                            nc.sync.dma_start(out=sh[0:64, :], in_=qkvT[cc][64:128, :])
                            nc.sync.dma_start(out=sh[64:128, :], in_=qkvT[cc][0:64, :])
                            t1 = ropool.tile([HD, S], dtr, tag="t1")
                            nc.vector.tensor_mul(t1[:], qkvT[cc][:], cos_t[:])
                            nc.vector.tensor_mul(sh[:], sh[:], sinp_t[:])
                            nc.vector.tensor_add(qkvT[cc][:], t1[:], sh[:])

            # ---------------- Phase C: V natural + ones column ----------------
            with tc.tile_pool(name="psC", bufs=2, space="PSUM") as psC:
                for kv in range(KVL):
                    for kt in range(TT):
                        pt = psC.tile([128, 128], dtr, tag="ptC")
                        nc.tensor.transpose(
                            pt[:], qkvT[HL + KVL + kv][:, kt * 128:(kt + 1) * 128], ident[:])
                        nc.vector.tensor_copy(v_nat[kv][kt][:, 0:HD], pt[:])
                        nc.vector.tensor_copy(v_nat[kv][kt][:, HD:HD + 1], ones_f[:])

            # ---------------- Phase D: attention per head ----------------
            with tc.tile_pool(name="pT", bufs=12) as ptpool, \
                 tc.tile_pool(name="ynorm", bufs=3) as ypool, \
                 tc.tile_pool(name="recs", bufs=3) as recpool, \
                 tc.tile_pool(name="psS", bufs=4, space="PSUM") as psS, \
                 tc.tile_pool(name="psY", bufs=2, space="PSUM") as psY, \
                 tc.tile_pool(name="psYT", bufs=2, space="PSUM") as psYT:
                for h in range(HL):
                    kv = h // (HL // KVL)
                    kT = qkvT[HL + kv]
                    pts = []
                    for kc in range(TT):
                        pt = ptpool.tile([128, S], dtr, tag="pT")
                        for tb in range(2):
                            sp = psS.tile([128, 512], dt, tag="sp")
                            nc.tensor.matmul(
                                sp[:],
                                kT[:, kc * 128:(kc + 1) * 128],
                                qkvT[h][:, tb * 512:(tb + 1) * 512],
                                start=True, stop=True)
                            nc.scalar.activation(pt[:, tb * 512:(tb + 1) * 512], sp[:],
                                                 mybir.ActivationFunctionType.Exp,
                                                 scale=float(SCALE))
                        pts.append(pt)
                    for qt in range(TT):
                        yp = psY.tile([128, HD + 1], dt, tag="yp")
                        for kc in range(TT):
                            nc.tensor.matmul(
                                yp[:], pts[kc][:, qt * 128:(qt + 1) * 128],
                                v_nat[kv][kc][:],
                                start=(kc == 0), stop=(kc == TT - 1))
                        rec = recpool.tile([128, 1], dt, tag="rec")
                        nc.vector.reciprocal(rec[:], yp[:, HD:HD + 1])
                        ysb = ypool.tile([128, HD], dtr, tag="ysb")
                        nc.vector.tensor_scalar_mul(ysb[:], yp[:, 0:HD], rec[:])
                        ytp = psYT.tile([128, 128], dtr, tag="ytp")
                        nc.tensor.transpose(ytp[:], ysb[:], ident[:])
                        nc.vector.tensor_copy(yT[h][:, qt * 128:(qt + 1) * 128], ytp[:])

            # ---------------- Phase E: out projection (partial, transposed) ----------------
            with tc.tile_pool(name="wo", bufs=3) as wopool, \
                 tc.tile_pool(name="osb", bufs=3) as opool, \
                 tc.tile_pool(name="psO", bufs=2, space="PSUM") as psO:
                for oc in range(E // 128):
                    op = psO.tile([128, S], dt, tag="op")
                    wt = wopool.tile([128, HL, 128], dtr, tag="wo")
                    nc.sync.dma_start(
                        out=wt[:],
                        in_=wo_d[:, oc * 128:(oc + 1) * 128].rearrange(
                            "(c p) m -> p c m", p=128))
                    for yc in range(HL):
                        for tb in range(2):
                            nc.tensor.matmul(
                                op[:, tb * 512:(tb + 1) * 512], wt[:, yc, :],
                                yT[yc][:, tb * 512:(tb + 1) * 512],
                                start=(yc == 0), stop=(yc == HL - 1))
                    ot = opool.tile([128, S], dt, tag="ot")
                    nc.scalar.copy(ot[:], op[:])
                    nc.sync.dma_start(
                        out=out_d[oc * 128:(oc + 1) * 128, :], in_=ot[:])

    nc.compile()
    return nc


def _rope_tables():
    inv = 1.0 / (10000.0 ** (np.arange(0, HD, 2, dtype=np.float32) / HD))  # [64]
    ang = np.arange(S, dtype=np.float32)[None, :] * inv[:, None]           # [64, S]
    cos = np.concatenate([np.cos(ang), np.cos(ang)], axis=0).astype(np.float32)   # [128, S]
    sin = np.sin(ang)
    sinp = np.concatenate([-sin, sin], axis=0).astype(np.float32)          # [128, S]
    return cos, sinp


def kernel(x, wq, wk, wv, wo):
    global _PROGRAM
    from concourse.bass_utils import run_bass_kernel_spmd

    if _PROGRAM is None:
        _PROGRAM = _build_program()
    nc = _PROGRAM

    cos, sinp = _rope_tables()
    ndt = np.float16 if MM_DT == "float16" else np.float32
    x = np.ascontiguousarray(x, dtype=np.float32)
    in_maps = []
    for c in range(N_CORES):
        b, r = c // TP, c % TP
        in_maps.append({
            "xt": np.ascontiguousarray(x[b].T).astype(ndt),
            "wq": np.ascontiguousarray(wq[:, r * QCOLS:(r + 1) * QCOLS], dtype=ndt),
            "wk": np.ascontiguousarray(wk[:, r * KVCOLS:(r + 1) * KVCOLS], dtype=ndt),
            "wv": np.ascontiguousarray(wv[:, r * KVCOLS:(r + 1) * KVCOLS], dtype=ndt),
            "wo": np.ascontiguousarray(wo[r * QCOLS:(r + 1) * QCOLS, :], dtype=ndt),
            "cos": cos.astype(ndt),
            "sinp": sinp.astype(ndt),
        })

    res = run_bass_kernel_spmd(nc, in_maps, list(range(N_CORES)))

    out = np.zeros((B, S, E), dtype=np.float32)
    for c in range(N_CORES):
        b = c // TP
        out[b] += res.results[c]["out_t"].T
    return out



# revision 2
# speedup vs baseline: 1.1945x; 1.1945x over previous
"""GQA (B=2,S=1024,E=4096,H=32,KV=8,HD=128, RoPE, no causal mask) on 8 NeuronCores.

Sharding: 2 batch-groups x 4-way head tensor-parallel.
Core c: batch b=c//4, tp rank r=c%4 -> 8 q heads [8r,8r+8), 2 kv heads [2r,2r+2),
wo rows [1024r, 1024(r+1)).  Each core computes a partial output
out_part = y_local @ wo[local_rows, :]  (emitted transposed as [4096, 1024] fp16);
host sums the 4 partials per batch. No device collectives needed.

v2: single fused pipeline. Projections are chunk-major (full-E accumulation in
PSUM), ordered K0,K1,V0,V1,Q0..Q7.  Head h's QK matmuls + exp are interleaved
into head h+1's Q projection so the scalar-engine exp work (~110us) hides under
Tensor work; PV for head h runs two iterations later.  Output DMA is fp16.
"""
import sys

sys.path.insert(0, "/opt/trn_rl_repo")

import numpy as np

B = 2
S = 1024
E = 4096
HD = 128
N_CORES = 8
TP = 4            # tensor-parallel ranks per batch group
HL = 8            # q heads per core
KVL = 2           # kv heads per core
QCOLS = HL * HD   # 1024
KVCOLS = KVL * HD  # 256
ECH = E // 128    # 32 e-chunks
TT = S // 128     # 8 token tiles
SCALE = 1.0 / np.sqrt(np.float32(HD))
MM_DT = "float16"

_PROGRAM = None


def _build_program():
    import concourse.bass as bass  # noqa: F401
    from concourse import bacc
    import concourse.mybir as mybir
    from concourse.tile import TileContext
    from concourse.masks import make_identity

    dt = mybir.dt.float32
    dtr = getattr(mybir.dt, MM_DT)
    nc = bacc.Bacc("TRN2", target_bir_lowering=False, debug=False,
                   num_devices=N_CORES)

    xt_d = nc.declare_dram_parameter("xt", [E, S], dtr, isOutput=False)
    wq_d = nc.declare_dram_parameter("wq", [E, QCOLS], dtr, isOutput=False)
    wk_d = nc.declare_dram_parameter("wk", [E, KVCOLS], dtr, isOutput=False)
    wv_d = nc.declare_dram_parameter("wv", [E, KVCOLS], dtr, isOutput=False)
    wo_d = nc.declare_dram_parameter("wo", [QCOLS, E], dtr, isOutput=False)
    cos_d = nc.declare_dram_parameter("cos", [HD, S], dtr, isOutput=False)
    sinp_d = nc.declare_dram_parameter("sinp", [HD, S], dtr, isOutput=False)
    out_d = nc.declare_dram_parameter("out_t", [E, S], dtr, isOutput=True)

    def w_src(cc):
        # full [4096, 128] column block for chunk cc, as [128, 32, 128]
        if cc < HL:
            return wq_d[:, cc * 128:(cc + 1) * 128]
        if cc < HL + KVL:
            return wk_d[:, (cc - HL) * 128:(cc - HL + 1) * 128]
        return wv_d[:, (cc - HL - KVL) * 128:(cc - HL - KVL + 1) * 128]

    with TileContext(nc) as tc:
        with tc.tile_pool(name="const", bufs=1) as cpool, \
             tc.tile_pool(name="persist", bufs=1) as ppool, \
             tc.tile_pool(name="vnat", bufs=1) as vpool, \
             tc.tile_pool(name="wstream", bufs=3) as wpool, \
             tc.tile_pool(name="rope", bufs=2) as ropool:
            ident_f = cpool.tile([128, 128], dt)
            make_identity(nc, ident_f[:])
            ident = cpool.tile([128, 128], dtr)
            nc.scalar.copy(ident[:], ident_f[:])
            cos_t = cpool.tile([HD, S], dtr, tag="cos")
            sinp_t = cpool.tile([HD, S], dtr, tag="sinp")
            nc.sync.dma_start(out=cos_t[:], in_=cos_d[:])
            nc.sync.dma_start(out=sinp_t[:], in_=sinp_d[:])

            # persistent data
            xs = ppool.tile([128, ECH, S], dtr, tag="xs", name="xs")
            kT = [ppool.tile([128, S], dtr, tag=f"kT{i}", name=f"kT{i}")
                  for i in range(KVL)]
            yT = [ppool.tile([128, S], dtr, tag=f"yT{i}", name=f"yT{i}")
                  for i in range(HL)]
            v_nat = [[vpool.tile([128, HD + 1], dtr, tag=f"v{kv}_{kt}",
                                 name=f"v{kv}_{kt}")
                      for kt in range(TT)] for kv in range(KVL)]

            # weight DMAs for the first two chunks (K0, K1) ahead of x so the
            # first matmuls are not stuck behind 8.4MB of x DMA
            def w_dma(cc, name):
                wt = wpool.tile([128, ECH, 128], dtr, tag="w", name=name)
                src = w_src(cc).rearrange("(c p) m -> p c m", p=128)
                for es in range(4):
                    nc.sync.dma_start(out=wt[:, es * 8:(es + 1) * 8, :],
                                      in_=src[:, es * 8:(es + 1) * 8, :])
                return wt

            wt_k0 = w_dma(HL + 0, "wt_k0")
            wt_k1 = w_dma(HL + 1, "wt_k1")
            for ec in range(ECH):
                nc.sync.dma_start(out=xs[:, ec, :],
                                  in_=xt_d[ec * 128:(ec + 1) * 128, :])

            def proj_chunk(psum_pool, wt, acc_bufs, interleave=None):
                acc = psum_pool.tile([128, S], dt, tag="acc", bufs=acc_bufs,
                                     name="acc")
                for ec in range(ECH):
                    for tb in range(2):
                        nc.tensor.matmul(
                            acc[:, tb * 512:(tb + 1) * 512], wt[:, ec, :],
                            xs[:, ec, tb * 512:(tb + 1) * 512],
                            start=(ec == 0), stop=(ec == ECH - 1),
                            skip_group_check=True)
                    if interleave is not None and ec % 4 == 3:
                        interleave(ec // 4)
                return acc

            def rope(dstT, acc):
                tmp = ropool.tile([HD, S], dtr, tag="t0", name="tmp")
                nc.scalar.copy(tmp[:, 0:512], acc[:, 0:512])
                nc.scalar.copy(tmp[:, 512:S], acc[:, 512:S])
                sh = ropool.tile([HD, S], dtr, tag="sh", name="sh")
                nc.sync.dma_start(out=sh[0:64, :], in_=tmp[64:128, :])
                nc.sync.dma_start(out=sh[64:128, :], in_=tmp[0:64, :])
                t1 = ropool.tile([HD, S], dtr, tag="t1", name="t1")
                nc.vector.tensor_mul(t1[:], tmp[:], cos_t[:])
                nc.vector.tensor_mul(sh[:], sh[:], sinp_t[:])
                nc.vector.tensor_add(dstT[:], t1[:], sh[:])

            # ---------------- pre-head phase: K and V chunks ----------------
            with tc.tile_pool(name="psPre", bufs=1, space="PSUM") as psPre:
                for i in range(KVL):
                    wt = wt_k0 if i == 0 else wt_k1
                    acc = proj_chunk(psPre, wt, acc_bufs=2)
                    rope(kT[i], acc)
                for i in range(KVL):
                    wt = w_dma(HL + KVL + i, f"wt_v{i}")
                    acc = proj_chunk(psPre, wt, acc_bufs=2)
                    vtmp = ropool.tile([128, S], dtr, tag="t0", name="vtmp")
                    nc.scalar.copy(vtmp[:, 0:512], acc[:, 0:512])
                    nc.scalar.copy(vtmp[:, 512:S], acc[:, 512:S])
                    for kt in range(TT):
                        pt = psPre.tile([128, 128], dtr, tag="small", bufs=2,
                                        name="vtp")
                        nc.tensor.transpose(
                            pt[:], vtmp[:, kt * 128:(kt + 1) * 128], ident[:])
                        nc.vector.tensor_copy(v_nat[i][kt][:, 0:HD], pt[:])
                        nc.vector.memset(v_nat[i][kt][:, HD:HD + 1], 1.0)

            # ---------------- head loop ----------------
            with tc.tile_pool(name="psProj", bufs=1, space="PSUM") as psProj, \
                 tc.tile_pool(name="psS", bufs=2, space="PSUM") as psS, \
                 tc.tile_pool(name="psSmall", bufs=4, space="PSUM") as psSm, \
                 tc.tile_pool(name="qroll", bufs=3) as qpool, \
                 tc.tile_pool(name="pt", bufs=18) as ptpool, \
                 tc.tile_pool(name="ynorm", bufs=3) as ypool, \
                 tc.tile_pool(name="recs", bufs=3) as recpool:
                qT = [None] * HL
                pts = [None] * HL

                def qk_pair(h, kc):
                    kv = h // (HL // KVL)
                    if kc == 0:
                        pts[h] = [ptpool.tile([128, S], dtr, tag="pt",
                                              name=f"pt{h}_{k}")
                                  for k in range(TT)]
                    for tb in range(2):
                        sp = psS.tile([128, 512], dt, tag="sp", name="sp")
                        nc.tensor.matmul(
                            sp[:], kT[kv][:, kc * 128:(kc + 1) * 128],
                            qT[h][:, tb * 512:(tb + 1) * 512],
                            start=True, stop=True, skip_group_check=True)
                        nc.scalar.activation(
                            pts[h][kc][:, tb * 512:(tb + 1) * 512], sp[:],
                            mybir.ActivationFunctionType.Exp,
                            scale=float(SCALE))

                def pv_block(h, qt):
                    kv = h // (HL // KVL)
                    yp = psSm.tile([128, HD + 1], dt, tag="small", name="yp")
                    for kc in range(TT):
                        nc.tensor.matmul(
                            yp[:], pts[h][kc][:, qt * 128:(qt + 1) * 128],
                            v_nat[kv][kc][:],
                            start=(kc == 0), stop=(kc == TT - 1),
                            skip_group_check=True)
                    rec = recpool.tile([128, 1], dt, tag="rec", name="rec")
                    nc.vector.reciprocal(rec[:], yp[:, HD:HD + 1])
                    ysb = ypool.tile([128, HD], dtr, tag="ysb", name="ysb")
                    nc.vector.tensor_scalar_mul(ysb[:], yp[:, 0:HD], rec[:])
                    ytp = psSm.tile([128, 128], dtr, tag="small", name="ytp")
                    nc.tensor.transpose(ytp[:], ysb[:], ident[:])
                    nc.vector.tensor_copy(yT[h][:, qt * 128:(qt + 1) * 128],
                                          ytp[:])

                for it in range(HL):
                    wt = w_dma(it, f"wt_q{it}")
                    interleave = (lambda kc, h=it - 1: qk_pair(h, kc)) \
                        if it >= 1 else None
                    acc = proj_chunk(psProj, wt, acc_bufs=1,
                                     interleave=interleave)
                    qT[it] = qpool.tile([128, S], dtr, tag="qT",
                                        name=f"qT{it}")
                    rope(qT[it], acc)
                    if it >= 2:
                        for qt in range(TT):
                            pv_block(it - 2, qt)
                # tail: QK for head 7 interleaved with PV for head 6
                for kc in range(TT):
                    qk_pair(HL - 1, kc)
                    pv_block(HL - 2, kc)
                for qt in range(TT):
                    pv_block(HL - 1, qt)

            # ---------------- out projection (partial, transposed, fp16) ----
            with tc.tile_pool(name="wo", bufs=3) as wopool, \
                 tc.tile_pool(name="osb", bufs=3) as opool, \
                 tc.tile_pool(name="psO", bufs=2, space="PSUM") as psO:
                for oc in range(E // 128):
                    wt = wopool.tile([128, HL, 128], dtr, tag="wo", name="wt_o")
                    nc.sync.dma_start(
                        out=wt[:],
                        in_=wo_d[:, oc * 128:(oc + 1) * 128].rearrange(
                            "(c p) m -> p c m", p=128))
                    op = psO.tile([128, S], dt, tag="op", name="op")
                    for yc in range(HL):
                        for tb in range(2):
                            nc.tensor.matmul(
                                op[:, tb * 512:(tb + 1) * 512], wt[:, yc, :],
                                yT[yc][:, tb * 512:(tb + 1) * 512],
                                start=(yc == 0), stop=(yc == HL - 1),
                                skip_group_check=True)
                    ot = opool.tile([128, S], dtr, tag="ot", name="ot")
                    nc.scalar.copy(ot[:, 0:512], op[:, 0:512])
                    nc.scalar.copy(ot[:, 512:S], op[:, 512:S])
                    nc.sync.dma_start(
                        out=out_d[oc * 128:(oc + 1) * 128, :], in_=ot[:])

    nc.compile()
    return nc


def _rope_tables():
    inv = 1.0 / (10000.0 ** (np.arange(0, HD, 2, dtype=np.float32) / HD))  # [64]
    ang = np.arange(S, dtype=np.float32)[None, :] * inv[:, None]           # [64, S]
    cos = np.concatenate([np.cos(ang), np.cos(ang)], axis=0).astype(np.float32)   # [128, S]
    sin = np.sin(ang)
    sinp = np.concatenate([-sin, sin], axis=0).astype(np.float32)          # [128, S]
    return cos, sinp


def kernel(x, wq, wk, wv, wo):
    global _PROGRAM
    from concourse.bass_utils import run_bass_kernel_spmd

    if _PROGRAM is None:
        _PROGRAM = _build_program()
    nc = _PROGRAM

    cos, sinp = _rope_tables()
    ndt = np.float16 if MM_DT == "float16" else np.float32
    x = np.ascontiguousarray(x, dtype=np.float32)
    in_maps = []
    for c in range(N_CORES):
        b, r = c // TP, c % TP
        in_maps.append({
            "xt": np.ascontiguousarray(x[b].T).astype(ndt),
            "wq": np.ascontiguousarray(wq[:, r * QCOLS:(r + 1) * QCOLS], dtype=ndt),
            "wk": np.ascontiguousarray(wk[:, r * KVCOLS:(r + 1) * KVCOLS], dtype=ndt),
            "wv": np.ascontiguousarray(wv[:, r * KVCOLS:(r + 1) * KVCOLS], dtype=ndt),
            "wo": np.ascontiguousarray(wo[r * QCOLS:(r + 1) * QCOLS, :], dtype=ndt),
            "cos": cos.astype(ndt),
            "sinp": sinp.astype(ndt),
        })

    res = run_bass_kernel_spmd(nc, in_maps, list(range(N_CORES)))

    out = np.zeros((B, S, E), dtype=np.float32)
    for c in range(N_CORES):
        b = c // TP
        out[b] += res.results[c]["out_t"].T.astype(np.float32)
    return out


# revision 7
# speedup vs baseline: 1.2021x; 1.0064x over previous
"""GQA (B=2,S=1024,E=4096,H=32,KV=8,HD=128, RoPE, no causal mask) on 8 NeuronCores.

Sharding: 2 batch-groups x 4-way head tensor-parallel.
Core c: batch b=c//4, tp rank r=c%4 -> 8 q heads [8r,8r+8), 2 kv heads [2r,2r+2),
wo rows [1024r, 1024(r+1)).  Each core computes a partial output
out_part = y_local @ wo[local_rows, :]  (emitted transposed as [4096, 1024] fp16);
host sums the 4 partials per batch. No device collectives needed.

v2: single fused pipeline. Projections are chunk-major (full-E accumulation in
PSUM), ordered K0,K1,V0,V1,Q0..Q7.  Head h's QK matmuls + exp are interleaved
into head h+1's Q projection so the scalar-engine exp work (~110us) hides under
Tensor work; PV for head h runs two iterations later.  Output DMA is fp16.
"""
import sys

sys.path.insert(0, "/opt/trn_rl_repo")

import numpy as np

B = 2
S = 1024
E = 4096
HD = 128
N_CORES = 8
TP = 4            # tensor-parallel ranks per batch group
HL = 8            # q heads per core
KVL = 2           # kv heads per core
QCOLS = HL * HD   # 1024
KVCOLS = KVL * HD  # 256
ECH = E // 128    # 32 e-chunks
TT = S // 128     # 8 token tiles
SCALE = 1.0 / np.sqrt(np.float32(HD))
MM_DT = "float16"

_PROGRAM = None


def _build_program():
    import concourse.bass as bass  # noqa: F401
    from concourse import bacc
    import concourse.mybir as mybir
    from concourse.tile import TileContext
    from concourse.masks import make_identity

    dt = mybir.dt.float32
    dtr = getattr(mybir.dt, MM_DT)
    nc = bacc.Bacc("TRN2", target_bir_lowering=False, debug=False,
                   num_devices=N_CORES)

    xt_d = nc.declare_dram_parameter("xt", [E, S], dtr, isOutput=False)
    wq_d = nc.declare_dram_parameter("wq", [E, QCOLS], dtr, isOutput=False)
    wk_d = nc.declare_dram_parameter("wk", [E, KVCOLS], dtr, isOutput=False)
    wv_d = nc.declare_dram_parameter("wv", [E, KVCOLS], dtr, isOutput=False)
    wo_d = nc.declare_dram_parameter("wo", [QCOLS, E], dtr, isOutput=False)
    cos_d = nc.declare_dram_parameter("cos", [HD, S], dtr, isOutput=False)
    sinp_d = nc.declare_dram_parameter("sinp", [HD, S], dtr, isOutput=False)
    out_d = nc.declare_dram_parameter("out_t", [E, S], dtr, isOutput=True)

    def w_src(cc):
        # full [4096, 128] column block for chunk cc, as [128, 32, 128]
        if cc < HL:
            return wq_d[:, cc * 128:(cc + 1) * 128]
        if cc < HL + KVL:
            return wk_d[:, (cc - HL) * 128:(cc - HL + 1) * 128]
        return wv_d[:, (cc - HL - KVL) * 128:(cc - HL - KVL + 1) * 128]

    with TileContext(nc) as tc:
        with tc.tile_pool(name="const", bufs=1) as cpool, \
             tc.tile_pool(name="persist", bufs=1) as ppool, \
             tc.tile_pool(name="vnat", bufs=1) as vpool, \
             tc.tile_pool(name="wstream", bufs=3) as wpool, \
             tc.tile_pool(name="rope", bufs=2) as ropool:
            ident_f = cpool.tile([128, 128], dt)
            make_identity(nc, ident_f[:])
            ident = cpool.tile([128, 128], dtr)
            nc.scalar.copy(ident[:], ident_f[:])
            cos_t = cpool.tile([HD, S], dtr, tag="cos")
            sinp_t = cpool.tile([HD, S], dtr, tag="sinp")
            nc.sync.dma_start(out=cos_t[:], in_=cos_d[:])
            nc.sync.dma_start(out=sinp_t[:], in_=sinp_d[:])

            # persistent data
            xs = ppool.tile([128, ECH, S], dtr, tag="xs", name="xs")
            kT = [ppool.tile([128, S], dtr, tag=f"kT{i}", name=f"kT{i}")
                  for i in range(KVL)]
            yT = [ppool.tile([128, S], dtr, tag=f"yT{i}", name=f"yT{i}")
                  for i in range(HL)]
            v_nat = [[vpool.tile([128, HD + 1], dtr, tag=f"v{kv}_{kt}",
                                 name=f"v{kv}_{kt}")
                      for kt in range(TT)] for kv in range(KVL)]

            # weight DMAs for the first two chunks (K0, K1) ahead of x so the
            # first matmuls are not stuck behind 8.4MB of x DMA
            def w_dma(cc, name):
                wt = wpool.tile([128, ECH, 128], dtr, tag="w", name=name)
                src = w_src(cc).rearrange("(c p) m -> p c m", p=128)
                for es in range(4):
                    nc.sync.dma_start(out=wt[:, es * 8:(es + 1) * 8, :],
                                      in_=src[:, es * 8:(es + 1) * 8, :])
                return wt

            wt_k0 = w_dma(HL + 0, "wt_k0")
            wt_k1 = w_dma(HL + 1, "wt_k1")
            for ec in range(ECH):
                nc.sync.dma_start(out=xs[:, ec, :],
                                  in_=xt_d[ec * 128:(ec + 1) * 128, :])

            def proj_chunk(psum_pool, wt, acc_bufs, interleave=None):
                acc = psum_pool.tile([128, S], dt, tag="acc", bufs=acc_bufs,
                                     name="acc")
                for ec in range(ECH):
                    for tb in range(2):
                        nc.tensor.matmul(
                            acc[:, tb * 512:(tb + 1) * 512], wt[:, ec, :],
                            xs[:, ec, tb * 512:(tb + 1) * 512],
                            start=(ec == 0), stop=(ec == ECH - 1),
                            skip_group_check=True)
                    if interleave is not None and ec % 4 == 3:
                        interleave(ec // 4)
                return acc

            def rope(dstT, acc):
                tmp = ropool.tile([HD, S], dtr, tag="t0", name="tmp")
                nc.scalar.copy(tmp[:, 0:512], acc[:, 0:512])
                nc.scalar.copy(tmp[:, 512:S], acc[:, 512:S])
                sh = ropool.tile([HD, S], dtr, tag="sh", name="sh")
                nc.sync.dma_start(out=sh[0:64, :], in_=tmp[64:128, :])
                nc.sync.dma_start(out=sh[64:128, :], in_=tmp[0:64, :])
                t1 = ropool.tile([HD, S], dtr, tag="t1", name="t1")
                nc.vector.tensor_mul(t1[:], tmp[:], cos_t[:])
                nc.vector.tensor_mul(sh[:], sh[:], sinp_t[:])
                nc.vector.tensor_add(dstT[:], t1[:], sh[:])

            # ---------------- pre-head phase: K and V chunks ----------------
            with tc.tile_pool(name="psPre", bufs=1, space="PSUM") as psPre:
                for i in range(KVL):
                    wt = wt_k0 if i == 0 else wt_k1
                    acc = proj_chunk(psPre, wt, acc_bufs=2)
                    rope(kT[i], acc)
                for i in range(KVL):
                    wt = w_dma(HL + KVL + i, f"wt_v{i}")
                    acc = proj_chunk(psPre, wt, acc_bufs=2)
                    vtmp = ropool.tile([128, S], dtr, tag="t0", name="vtmp")
                    nc.scalar.copy(vtmp[:, 0:512], acc[:, 0:512])
                    nc.scalar.copy(vtmp[:, 512:S], acc[:, 512:S])
                    for kt in range(TT):
                        pt = psPre.tile([128, 128], dtr, tag="small", bufs=2,
                                        name="vtp")
                        nc.tensor.transpose(
                            pt[:], vtmp[:, kt * 128:(kt + 1) * 128], ident[:])
                        nc.vector.tensor_copy(v_nat[i][kt][:, 0:HD], pt[:])
                        nc.vector.memset(v_nat[i][kt][:, HD:HD + 1], 1.0)

            # ---------------- head loop ----------------
            with tc.tile_pool(name="psProj", bufs=1, space="PSUM") as psProj, \
                 tc.tile_pool(name="psS", bufs=2, space="PSUM") as psS, \
                 tc.tile_pool(name="psSmall", bufs=4, space="PSUM") as psSm, \
                 tc.tile_pool(name="qroll", bufs=3) as qpool, \
                 tc.tile_pool(name="pt", bufs=18) as ptpool, \
                 tc.tile_pool(name="ynorm", bufs=3) as ypool, \
                 tc.tile_pool(name="recs", bufs=3) as recpool:
                qT = [None] * HL
                pts = [None] * HL

                def qk_pair(h, kc):
                    kv = h // (HL // KVL)
                    if kc == 0:
                        pts[h] = [ptpool.tile([128, S], dtr, tag="pt",
                                              name=f"pt{h}_{k}")
                                  for k in range(TT)]
                    for tb in range(2):
                        sp = psS.tile([128, 512], dt, tag="sp", name="sp")
                        nc.tensor.matmul(
                            sp[:], kT[kv][:, kc * 128:(kc + 1) * 128],
                            qT[h][:, tb * 512:(tb + 1) * 512],
                            start=True, stop=True, skip_group_check=True)
                        nc.scalar.activation(
                            pts[h][kc][:, tb * 512:(tb + 1) * 512], sp[:],
                            mybir.ActivationFunctionType.Exp,
                            scale=float(SCALE))

                def pv_mm(h, qt, yp_pool, yp_tag):
                    # PV accumulation + normalize for one 128-token q block
                    kv = h // (HL // KVL)
                    yp = yp_pool.tile([128, 512], dt, tag=yp_tag, name="yp")
                    for kc in range(TT):
                        nc.tensor.matmul(
                            yp[:, 0:HD + 1],
                            pts[h][kc][:, qt * 128:(qt + 1) * 128],
                            v_nat[kv][kc][:],
                            start=(kc == 0), stop=(kc == TT - 1),
                            skip_group_check=True)
                    rec = recpool.tile([128, 1], dt, tag="rec", name="rec")
                    nc.vector.reciprocal(rec[:], yp[:, HD:HD + 1])
                    ysb = ypool.tile([128, HD], dtr, tag="ysb", name="ysb")
                    nc.vector.tensor_scalar_mul(ysb[:], yp[:, 0:HD], rec[:])
                    return ysb

                def pv_fin(h, qt, ysb):
                    # transpose normalized y block into yT (deferred so the
                    # DVE normalize chain is hidden under other PE work)
                    ytp = psSm.tile([128, 128], dtr, tag="small", name="ytp")
                    nc.tensor.transpose(ytp[:], ysb[:], ident[:])
                    nc.vector.tensor_copy(yT[h][:, qt * 128:(qt + 1) * 128],
                                          ytp[:])

                pend = [None]  # pending (h, qt, ysb) awaiting pv_fin

                def head_step(it, g):
                    # one interleave step inside chunk `it`'s projection:
                    # QK pair g for head it-1, PV block g for head it-2
                    qk_pair(it - 1, g)
                    if pend[0] is not None:
                        pv_fin(*pend[0])
                        pend[0] = None
                    if it >= 2:
                        ysb = pv_mm(it - 2, g, psSm, "small")
                        pend[0] = (it - 2, g, ysb)

                for it in range(HL):
                    wt = w_dma(it, f"wt_q{it}")
                    interleave = (lambda g, it=it: head_step(it, g)) \
                        if it >= 1 else None
                    acc = proj_chunk(psProj, wt, acc_bufs=1,
                                     interleave=interleave)
                    qT[it] = qpool.tile([128, S], dtr, tag="qT",
                                        name=f"qT{it}")
                    rope(qT[it], acc)
                # tail: QK for head 7 interleaved with PV for head 6
                for g in range(TT):
                    qk_pair(HL - 1, g)
                    if pend[0] is not None:
                        pv_fin(*pend[0])
                    pend[0] = (HL - 2, g, pv_mm(HL - 2, g, psSm, "small"))
                # last head's PV: two-deep software pipeline, yp from the
                # (now free) score pool so transposes trail by two blocks
                fins = [pend[0]]
                pend[0] = None
                for qt in range(TT):
                    fins.append((HL - 1, qt, pv_mm(HL - 1, qt, psS, "sp")))
                    if len(fins) >= 3:
                        pv_fin(*fins.pop(0))
                for f in fins:
                    pv_fin(*f)

            # ---------------- out projection (partial, transposed, fp16) ----
            with tc.tile_pool(name="wo", bufs=3) as wopool, \
                 tc.tile_pool(name="osb", bufs=3) as opool, \
                 tc.tile_pool(name="psO", bufs=2, space="PSUM") as psO:
                for oc in range(E // 128):
                    wt = wopool.tile([128, HL, 128], dtr, tag="wo", name="wt_o")
                    nc.sync.dma_start(
                        out=wt[:],
                        in_=wo_d[:, oc * 128:(oc + 1) * 128].rearrange(
                            "(c p) m -> p c m", p=128))
                    op = psO.tile([128, S], dt, tag="op", name="op")
                    for yc in range(HL):
                        for tb in range(2):
                            nc.tensor.matmul(
                                op[:, tb * 512:(tb + 1) * 512], wt[:, yc, :],
                                yT[yc][:, tb * 512:(tb + 1) * 512],
                                start=(yc == 0), stop=(yc == HL - 1),
                                skip_group_check=True)
                    ot = opool.tile([128, S], dtr, tag="ot", name="ot")
                    nc.scalar.copy(ot[:, 0:512], op[:, 0:512])
                    nc.scalar.copy(ot[:, 512:S], op[:, 512:S])
                    nc.sync.dma_start(
                        out=out_d[oc * 128:(oc + 1) * 128, :], in_=ot[:])

    nc.compile()
    return nc


def _rope_tables():
    inv = 1.0 / (10000.0 ** (np.arange(0, HD, 2, dtype=np.float32) / HD))  # [64]
    ang = np.arange(S, dtype=np.float32)[None, :] * inv[:, None]           # [64, S]
    cos = np.concatenate([np.cos(ang), np.cos(ang)], axis=0).astype(np.float32)   # [128, S]
    sin = np.sin(ang)
    sinp = np.concatenate([-sin, sin], axis=0).astype(np.float32)          # [128, S]
    return cos, sinp


def kernel(x, wq, wk, wv, wo):
    global _PROGRAM
    from concourse.bass_utils import run_bass_kernel_spmd

    if _PROGRAM is None:
        _PROGRAM = _build_program()
    nc = _PROGRAM

    cos, sinp = _rope_tables()
    ndt = np.float16 if MM_DT == "float16" else np.float32
    x = np.ascontiguousarray(x, dtype=np.float32)
    in_maps = []
    for c in range(N_CORES):
        b, r = c // TP, c % TP
        in_maps.append({
            "xt": np.ascontiguousarray(x[b].T).astype(ndt),
            "wq": np.ascontiguousarray(wq[:, r * QCOLS:(r + 1) * QCOLS], dtype=ndt),
            "wk": np.ascontiguousarray(wk[:, r * KVCOLS:(r + 1) * KVCOLS], dtype=ndt),
            "wv": np.ascontiguousarray(wv[:, r * KVCOLS:(r + 1) * KVCOLS], dtype=ndt),
            "wo": np.ascontiguousarray(wo[r * QCOLS:(r + 1) * QCOLS, :], dtype=ndt),
            "cos": cos.astype(ndt),
            "sinp": sinp.astype(ndt),
        })

    res = run_bass_kernel_spmd(nc, in_maps, list(range(N_CORES)))

    out = np.zeros((B, S, E), dtype=np.float32)
    for c in range(N_CORES):
        b = c // TP
        out[b] += res.results[c]["out_t"].T.astype(np.float32)
    return out


# revision 13
# speedup vs baseline: 1.2217x; 1.0163x over previous
"""GQA (B=2,S=1024,E=4096,H=32,KV=8,HD=128, RoPE, no causal mask) on 8 NeuronCores.

Sharding: 2 batch-groups x 4-way head tensor-parallel.
Core c: batch b=c//4, tp rank r=c%4 -> 8 q heads [8r,8r+8), 2 kv heads [2r,2r+2),
wo rows [1024r, 1024(r+1)).  Each core computes a partial output
out_part = y_local @ wo[local_rows, :]  (emitted transposed as [4096, 1024] fp16);
host sums the 4 partials per batch. No device collectives needed.

v4: single fused pipeline.
- Projections are chunk-major (full-E accumulation in PSUM), order
  K0,K1,V0 interleaved per e-chunk (tracks the x DMA stream), V1, Q0..Q7.
- Head h's QK+exp / PV / y-transpose are slot-scheduled into chunk h+1 / h+2's
  projection groups so scalar-engine exp (~110us) and all DVE chains hide
  under Tensor work.
- Weights are host-prearranged so every weight DMA is contiguous per
  partition; output DMA is fp16 (host accumulates partials in fp32).
"""
import sys

sys.path.insert(0, "/opt/trn_rl_repo")

import numpy as np

B = 2
S = 1024
E = 4096
HD = 128
N_CORES = 8
TP = 4            # tensor-parallel ranks per batch group
HL = 8            # q heads per core
KVL = 2           # kv heads per core
QCOLS = HL * HD   # 1024
KVCOLS = KVL * HD  # 256
ECH = E // 128    # 32 e-chunks
TT = S // 128     # 8 token tiles
SCALE = 1.0 / np.sqrt(np.float32(HD))
MM_DT = "float16"

_PROGRAM = None


def _build_program():
    import concourse.bass as bass  # noqa: F401
    from concourse import bacc
    import concourse.mybir as mybir
    from concourse.tile import TileContext
    from concourse.masks import make_identity

    dt = mybir.dt.float32
    dtr = getattr(mybir.dt, MM_DT)
    nc = bacc.Bacc("TRN2", target_bir_lowering=False, debug=False,
                   num_devices=N_CORES)

    xt_d = nc.declare_dram_parameter("xt", [E, S], dtr, isOutput=False)
    # host-prearranged: row block cc*128+p holds w[:, cc*128:...] row c*128+p
    wq_d = nc.declare_dram_parameter("wq", [HL * 128, E], dtr, isOutput=False)
    wk_d = nc.declare_dram_parameter("wk", [KVL * 128, E], dtr, isOutput=False)
    wv_d = nc.declare_dram_parameter("wv", [KVL * 128, E], dtr, isOutput=False)
    wo_d = nc.declare_dram_parameter("wo", [ECH * 128, QCOLS], dtr,
                                     isOutput=False)
    cos_d = nc.declare_dram_parameter("cos", [HD, S], dtr, isOutput=False)
    sinp_d = nc.declare_dram_parameter("sinp", [HD, S], dtr, isOutput=False)
    out_d = nc.declare_dram_parameter("out_t", [E, S], dtr, isOutput=True)

    def w_src(cc):
        # [128, ECH, 128] view of chunk cc's weights, contiguous per partition
        if cc < HL:
            base = wq_d
        elif cc < HL + KVL:
            base, cc = wk_d, cc - HL
        else:
            base, cc = wv_d, cc - HL - KVL
        return base[cc * 128:(cc + 1) * 128, :].rearrange(
            "p (c m) -> p c m", m=128)

    with TileContext(nc) as tc:
        with tc.tile_pool(name="const", bufs=1) as cpool, \
             tc.tile_pool(name="persist", bufs=1) as ppool, \
             tc.tile_pool(name="vnat", bufs=1) as vpool, \
             tc.tile_pool(name="wstream", bufs=4) as wpool, \
             tc.tile_pool(name="rope", bufs=2) as ropool:
            ident_f = cpool.tile([128, 128], dt)
            make_identity(nc, ident_f[:])
            ident = cpool.tile([128, 128], dtr)
            nc.scalar.copy(ident[:], ident_f[:])
            cos_t = cpool.tile([HD, S], dtr, tag="cos")
            sinp_t = cpool.tile([HD, S], dtr, tag="sinp")

            # persistent data
            xs = ppool.tile([128, ECH, S], dtr, tag="xs", name="xs")
            kT = [ppool.tile([128, S], dtr, tag=f"kT{i}", name=f"kT{i}")
                  for i in range(KVL)]
            yT = [ppool.tile([128, S], dtr, tag=f"yT{i}", name=f"yT{i}")
                  for i in range(HL)]
            v_nat = [[vpool.tile([128, HD + 1], dtr, tag=f"v{kv}_{kt}",
                                 name=f"v{kv}_{kt}")
                      for kt in range(TT)] for kv in range(KVL)]

            # DMA emission order matters: the Sync engine issues descriptors
            # in order at ~240-330GB/s aggregate, so stage the first three
            # chunks' weights per-superchunk between x slices.
            pre_cc = [HL, HL + 1, HL + KVL]      # K0, K1, V0
            pre_wt = [wpool.tile([128, ECH, 128], dtr, tag="w",
                                 name=f"wt_pre{j}") for j in range(3)]
            wt_v1 = wpool.tile([128, ECH, 128], dtr, tag="w", name="wt_v1")
            for es in range(4):
                for j, cc in enumerate(pre_cc):
                    nc.sync.dma_start(
                        out=pre_wt[j][:, es * 8:(es + 1) * 8, :],
                        in_=w_src(cc)[:, es * 8:(es + 1) * 8, :])
                for ec in range(es * 8, (es + 1) * 8):
                    nc.sync.dma_start(out=xs[:, ec, :],
                                      in_=xt_d[ec * 128:(ec + 1) * 128, :])
                if es == 1:
                    nc.sync.dma_start(out=cos_t[:], in_=cos_d[:])
                    nc.sync.dma_start(out=sinp_t[:], in_=sinp_d[:])
                if es == 2:
                    nc.sync.dma_start(out=wt_v1[:],
                                      in_=w_src(HL + KVL + 1)[:])

            def w_dma(cc, name):
                wt = wpool.tile([128, ECH, 128], dtr, tag="w", name=name)
                nc.sync.dma_start(out=wt[:], in_=w_src(cc)[:])
                return wt

            def proj_chunk(psum_pool, wt, acc_bufs, interleave=None):
                acc = psum_pool.tile([128, S], dt, tag="acc", bufs=acc_bufs,
                                     name="acc")
                for ec in range(ECH):
                    for tb in range(2):
                        nc.tensor.matmul(
                            acc[:, tb * 512:(tb + 1) * 512], wt[:, ec, :],
                            xs[:, ec, tb * 512:(tb + 1) * 512],
                            start=(ec == 0), stop=(ec == ECH - 1),
                            skip_group_check=True)
                    if interleave is not None and ec % 4 == 3:
                        interleave(ec // 4)
                return acc

            def rope(dstT, acc):
                tmp = ropool.tile([HD, S], dtr, tag="t0", name="tmp")
                nc.scalar.copy(tmp[:, 0:512], acc[:, 0:512])
                nc.scalar.copy(tmp[:, 512:S], acc[:, 512:S])
                sh = ropool.tile([HD, S], dtr, tag="sh", name="sh")
                nc.sync.dma_start(out=sh[0:64, :], in_=tmp[64:128, :])
                nc.sync.dma_start(out=sh[64:128, :], in_=tmp[0:64, :])
                t1 = ropool.tile([HD, S], dtr, tag="t1", name="t1")
                nc.vector.tensor_mul(t1[:], tmp[:], cos_t[:])
                nc.vector.tensor_mul(sh[:], sh[:], sinp_t[:])
                nc.vector.tensor_add(dstT[:], t1[:], sh[:])

            # ---------------- pre-head phase ----------------
            # K0/K1/V0 interleaved per e-chunk so compute tracks the x DMA
            # stream; V1 follows once x is resident.
            with tc.tile_pool(name="psPre", bufs=1, space="PSUM") as psPre:
                pre_acc = [psPre.tile([128, S], dt, tag="acc", bufs=3,
                                      name=f"accp{j}") for j in range(3)]
                for ec in range(ECH):
                    for j in range(3):
                        for tb in range(2):
                            nc.tensor.matmul(
                                pre_acc[j][:, tb * 512:(tb + 1) * 512],
                                pre_wt[j][:, ec, :],
                                xs[:, ec, tb * 512:(tb + 1) * 512],
                                start=(ec == 0), stop=(ec == ECH - 1),
                                skip_group_check=True)
                rope(kT[0], pre_acc[0])
                acc_v1 = proj_chunk(psPre, wt_v1, acc_bufs=3)
                rope(kT[1], pre_acc[1])

                def v_flow(i, acc):
                    vtmp = ropool.tile([128, S], dtr, tag="t0", name="vtmp")
                    nc.scalar.copy(vtmp[:, 0:512], acc[:, 0:512])
                    nc.scalar.copy(vtmp[:, 512:S], acc[:, 512:S])
                    for kt in range(TT):
                        pt = psPre.tile([128, 128], dtr, tag="small", bufs=2,
                                        name="vtp")
                        nc.tensor.transpose(
                            pt[:], vtmp[:, kt * 128:(kt + 1) * 128], ident[:])
                        nc.vector.tensor_copy(v_nat[i][kt][:, 0:HD], pt[:])
                        nc.vector.memset(v_nat[i][kt][:, HD:HD + 1], 1.0)

                v_flow(0, pre_acc[2])
                v_flow(1, acc_v1)

            # ---------------- head loop ----------------
            # slot schedule inside chunk `it`'s projection, group g (0..7):
            #   g=0:  fin(it-3, 6), pv(it-3, 7), qk(it-1, 0)
            #   g=1:  fin(it-3, 7), pv(it-2, 0), qk(it-1, 1)
            #   g>=2: fin(it-2, g-2), pv(it-2, g-1), qk(it-1, g)
            # so every transpose (fin) trails its PV block by two groups and
            # exp for head it-1 is paced across the whole chunk.
            with tc.tile_pool(name="psProj", bufs=1, space="PSUM") as psProj, \
                 tc.tile_pool(name="psS", bufs=2, space="PSUM") as psS, \
                 tc.tile_pool(name="psSmall", bufs=4, space="PSUM") as psSm, \
                 tc.tile_pool(name="qroll", bufs=3) as qpool, \
                 tc.tile_pool(name="pt", bufs=20) as ptpool, \
                 tc.tile_pool(name="ynorm", bufs=3) as ypool, \
                 tc.tile_pool(name="recs", bufs=3) as recpool:
                qT = [None] * HL
                pts = [[None] * TT for _ in range(HL)]
                ysbs = {}

                def qk_pair(h, kc):
                    kv = h // (HL // KVL)
                    pts[h][kc] = ptpool.tile([128, S], dtr, tag="pt",
                                             name=f"pt{h}_{kc}")
                    for tb in range(2):
                        sp = psS.tile([128, 512], dt, tag="sp", name="sp")
                        nc.tensor.matmul(
                            sp[:], kT[kv][:, kc * 128:(kc + 1) * 128],
                            qT[h][:, tb * 512:(tb + 1) * 512],
                            start=True, stop=True, skip_group_check=True)
                        nc.scalar.activation(
                            pts[h][kc][:, tb * 512:(tb + 1) * 512], sp[:],
                            mybir.ActivationFunctionType.Exp,
                            scale=float(SCALE))

                def pv_mm(h, qt):
                    kv = h // (HL // KVL)
                    yp = psSm.tile([128, 512], dt, tag="small", name="yp")
                    for kc in range(TT):
                        nc.tensor.matmul(
                            yp[:, 0:HD + 1],
                            pts[h][kc][:, qt * 128:(qt + 1) * 128],
                            v_nat[kv][kc][:],
                            start=(kc == 0), stop=(kc == TT - 1),
                            skip_group_check=True)
                    rec = recpool.tile([128, 1], dt, tag="rec", name="rec")
                    nc.vector.reciprocal(rec[:], yp[:, HD:HD + 1])
                    ysb = ypool.tile([128, HD], dtr, tag="ysb", name="ysb")
                    nc.vector.tensor_scalar_mul(ysb[:], yp[:, 0:HD], rec[:])
                    ysbs[(h, qt)] = ysb

                def pv_fin(h, qt):
                    ysb = ysbs.pop((h, qt))
                    ytp = psSm.tile([128, 128], dtr, tag="small", name="ytp")
                    nc.tensor.transpose(ytp[:], ysb[:], ident[:])
                    nc.vector.tensor_copy(yT[h][:, qt * 128:(qt + 1) * 128],
                                          ytp[:])

                def head_step(it, g):
                    if g == 0:
                        fin_h, fin_qt = it - 3, 6
                        pv_h, pv_qt = it - 3, 7
                    elif g == 1:
                        fin_h, fin_qt = it - 3, 7
                        pv_h, pv_qt = it - 2, 0
                    else:
                        fin_h, fin_qt = it - 2, g - 2
                        pv_h, pv_qt = it - 2, g - 1
                    if 0 <= fin_h < HL and (fin_h, fin_qt) in ysbs:
                        pv_fin(fin_h, fin_qt)
                    if 0 <= pv_h < HL and pv_qt < TT:
                        pv_mm(pv_h, pv_qt)
                    if 0 <= it - 1 < HL and g < TT:
                        qk_pair(it - 1, g)

                for it in range(HL):
                    wt = w_dma(it, f"wt_q{it}")
                    interleave = (lambda g, it=it: head_step(it, g)) \
                        if it >= 1 else None
                    acc = proj_chunk(psProj, wt, acc_bufs=1,
                                     interleave=interleave)
                    qT[it] = qpool.tile([128, S], dtr, tag="qT",
                                        name=f"qT{it}")
                    rope(qT[it], acc)
                # tail: virtual iterations drain the remaining QK/PV/fin work
                for it in (HL, HL + 1, HL + 2):
                    for g in range(TT if it <= HL + 1 else 2):
                        head_step(it, g)

            # ---------------- out projection (partial, transposed, fp16) ----
            with tc.tile_pool(name="wo", bufs=3) as wopool, \
                 tc.tile_pool(name="osb", bufs=3) as opool, \
                 tc.tile_pool(name="psO", bufs=2, space="PSUM") as psO:
                for oc in range(E // 128):
                    wt = wopool.tile([128, HL, 128], dtr, tag="wo",
                                     name="wt_o")
                    nc.sync.dma_start(
                        out=wt[:],
                        in_=wo_d[oc * 128:(oc + 1) * 128, :].rearrange(
                            "p (c m) -> p c m", m=128))
                    op = psO.tile([128, S], dt, tag="op", name="op")
                    for yc in range(HL):
                        for tb in range(2):
                            nc.tensor.matmul(
                                op[:, tb * 512:(tb + 1) * 512], wt[:, yc, :],
                                yT[yc][:, tb * 512:(tb + 1) * 512],
                                start=(yc == 0), stop=(yc == HL - 1),
                                skip_group_check=True)
                    ot = opool.tile([128, S], dtr, tag="ot", name="ot")
                    nc.scalar.copy(ot[:, 0:512], op[:, 0:512])
                    nc.scalar.copy(ot[:, 512:S], op[:, 512:S])
                    nc.sync.dma_start(
                        out=out_d[oc * 128:(oc + 1) * 128, :], in_=ot[:])

    nc.compile()
    return nc


def _rope_tables():
    inv = 1.0 / (10000.0 ** (np.arange(0, HD, 2, dtype=np.float32) / HD))  # [64]
    ang = np.arange(S, dtype=np.float32)[None, :] * inv[:, None]           # [64, S]
    cos = np.concatenate([np.cos(ang), np.cos(ang)], axis=0).astype(np.float32)   # [128, S]
    sin = np.sin(ang)
    sinp = np.concatenate([-sin, sin], axis=0).astype(np.float32)          # [128, S]
    return cos, sinp


def _rearrange_w(w, n_chunks):
    # [E_rows, n_chunks*128] -> [n_chunks*128, E_rows] blocks: row cc*128+p
    # holds w[c*128+p, cc*128+m] at col c*128+m
    e_rows = w.shape[0]
    c = e_rows // 128
    return np.ascontiguousarray(
        w.reshape(c, 128, n_chunks, 128).transpose(2, 1, 0, 3).reshape(
            n_chunks * 128, e_rows))


def make_in_maps(x, wq, wk, wv, wo):
    cos, sinp = _rope_tables()
    ndt = np.float16 if MM_DT == "float16" else np.float32
    x = np.ascontiguousarray(x, dtype=np.float32)
    in_maps = []
    for c in range(N_CORES):
        b, r = c // TP, c % TP
        in_maps.append({
            "xt": np.ascontiguousarray(x[b].T).astype(ndt),
            "wq": _rearrange_w(
                wq[:, r * QCOLS:(r + 1) * QCOLS].astype(ndt), HL),
            "wk": _rearrange_w(
                wk[:, r * KVCOLS:(r + 1) * KVCOLS].astype(ndt), KVL),
            "wv": _rearrange_w(
                wv[:, r * KVCOLS:(r + 1) * KVCOLS].astype(ndt), KVL),
            "wo": _rearrange_w(
                wo[r * QCOLS:(r + 1) * QCOLS, :].astype(ndt), ECH),
            "cos": cos.astype(ndt),
            "sinp": sinp.astype(ndt),
        })
    return in_maps


def kernel(x, wq, wk, wv, wo):
    global _PROGRAM
    from concourse.bass_utils import run_bass_kernel_spmd

    if _PROGRAM is None:
        _PROGRAM = _build_program()
    nc = _PROGRAM

    res = run_bass_kernel_spmd(nc, make_in_maps(x, wq, wk, wv, wo),
                               list(range(N_CORES)))

    out = np.zeros((B, S, E), dtype=np.float32)
    for c in range(N_CORES):
        b = c // TP
        out[b] += res.results[c]["out_t"].T.astype(np.float32)
    return out
